# revision 1
# baseline (speedup 1.0000x reference)
"""Trainium2 Bass kernel for nn_Attention_41704132444382.

Masked-linear QKV projection + 16-head attention + masked-linear output
projection, tensor-parallel over heads across 8 NeuronCores (2 heads/core).

Layout strategy (all chosen to avoid on-device transposes of activations):
  - x is passed host-transposed as xT [1024, 4096] (k on partitions).
  - Q^T, K^T computed as [128 (2 heads x 64 d), 4096 t]  (d on partitions).
  - V^T computed the same way, then PE-transposed to V [t, dv] tiles with a
    ones column appended -> PV matmul yields attnout^T AND the softmax
    denominator (colsum) in one accumulation chain (M=65).
  - Scores computed as S^T [j keys on partitions, i queries free]; exp runs
    on ScalarE directly from PSUM with the 1/32 scale folded in (no max
    subtraction needed: |scores/32| <= ~7 so exp is safe in fp32).
  - Normalization: colsum rows are PE-transpose-gathered onto partitions,
    reciprocal on DVE, PE K=1-broadcast back to [64, i], fused into the
    PSUM->SBUF evacuation of attnout^T via tensor_tensor mult.
  - Output projection consumes attnT (dv on partitions) as lhsT directly;
    per-core partial outputs are summed on host; bias applied on host.

Matmuls use float32r (fp32 storage, 1 cyc/row on PE when N>=256 vs fp32's 4).
Set BASS_ATTN_F32R=0 to fall back to plain fp32 matmuls.
"""

import os
import sys

import numpy as np

sys.path.insert(0, "/opt/trn_rl_repo")

import concourse.bass as bass
import concourse.mybir as mybir
from concourse import bacc
from concourse.masks import make_identity
from concourse.tile import TileContext

DIM = 1024
HEADS = 16
B = 2
N = 2048
T = B * N  # 4096 flattened tokens
NCORES = 8
HPC = HEADS // NCORES  # 2 heads per core
DV = HPC * 64  # 128 head-dims per core
SCALE = DIM ** (-0.5)  # 1/32

F32 = mybir.dt.float32
F32R = mybir.dt.float32r

# matmul-operand dtype: "f32r" (default), "f32", or "bf16"
MM_DTYPE_NAME = os.environ.get("BASS_ATTN_MM_DTYPE", "f32r")
DT_MM = {"f32r": mybir.dt.float32r, "f32": F32, "bf16": mybir.dt.bfloat16}[MM_DTYPE_NAME]


def build_nc():
    nc = bacc.Bacc("TRN2", target_bir_lowering=True)
    xT_d = nc.declare_dram_parameter("xT", [DIM, T], F32, isOutput=False)
    wqkvT_d = nc.declare_dram_parameter("wqkvT", [DIM, 384], F32, isOutput=False)
    mqkvT_d = nc.declare_dram_parameter("mqkvT", [DIM, 384], F32, isOutput=False)
    woT_d = nc.declare_dram_parameter("woT", [DV, DIM], F32, isOutput=False)
    moT_d = nc.declare_dram_parameter("moT", [DV, DIM], F32, isOutput=False)
    out_d = nc.declare_dram_parameter("out", [T, DIM], F32, isOutput=True)

    gt = mybir.AluOpType.is_gt
    mult = mybir.AluOpType.mult
    Exp = mybir.ActivationFunctionType.Exp

    with TileContext(nc) as tc:
        with tc.tile_pool(name="persist", bufs=1) as pp:
            wqkv_g = pp.tile([128, 8 * 384], DT_MM)  # [k-part, (kt, o)]
            wo_g = pp.tile([128, 1024], DT_MM)
            qT = pp.tile([128, 4096], DT_MM)
            kTt = pp.tile([128, 4096], DT_MM)
            v1 = pp.tile([128, 32 * 65], DT_MM)  # [t-part, (jt, dv|1)] head 1
            v2 = pp.tile([128, 32 * 65], DT_MM)
            attnT = [pp.tile([128, 2048], DT_MM, name=f"attnT{bb}") for bb in range(B)]
            ident = pp.tile([128, 128], F32)
            ones1 = pp.tile([1, 64], DT_MM)

            make_identity(nc, ident[:])
            # memset can't emit float32r; memset f32 then cast-copy
            ones_f = pp.tile([128, 64], F32)
            nc.vector.memset(ones_f[:], 1.0)
            nc.vector.tensor_copy(ones1[:], ones_f[0:1, :])
            ones32 = pp.tile([128, 32], DT_MM)
            nc.vector.tensor_copy(ones32[:], ones_f[:, 0:32])
            # ones column at slot 64 of each 65-wide block of v1/v2 (strided write);
            # V evacuations only write cols 0..63 of each block.
            for vv in (v1, v2):
                nc.vector.tensor_copy(
                    vv[:].rearrange("p (j c) -> p j c", c=65)[:, :, 64:65],
                    ones32[:].rearrange("p (j c) -> p j c", c=1),
                )

            # ---------- Phase 0: load + gate weights ----------
            with tc.tile_pool(name="wload", bufs=2) as wl:
                wraw = wl.tile([128, 8 * 384], F32, tag="w")
                mraw = wl.tile([128, 8 * 384], F32, tag="w")
                g = wl.tile([128, 8 * 384], F32, tag="g")
                nc.sync.dma_start(
                    wraw[:].rearrange("p (kt o) -> p kt o", kt=8),
                    wqkvT_d[:].rearrange("(kt p) o -> p kt o", p=128),
                )
                nc.sync.dma_start(
                    mraw[:].rearrange("p (kt o) -> p kt o", kt=8),
                    mqkvT_d[:].rearrange("(kt p) o -> p kt o", p=128),
                )
                nc.vector.tensor_scalar(g[:], mraw[:], 0.0, None, gt)
                nc.vector.tensor_tensor(wqkv_g[:], wraw[:], g[:], mult)

                wor = wl.tile([128, 1024], F32, tag="wo")
                mor = wl.tile([128, 1024], F32, tag="wo")
                go = wl.tile([128, 1024], F32, tag="go")
                nc.sync.dma_start(wor[:], woT_d[:])
                nc.sync.dma_start(mor[:], moT_d[:])
                nc.vector.tensor_scalar(go[:], mor[:], 0.0, None, gt)
                nc.vector.tensor_tensor(wo_g[:], wor[:], go[:], mult)

            # ---------- Phase 1: QKV projection (+ V^T transpose) ----------
            vT = pp.tile([128, 4096], F32)
            with (
                tc.tile_pool(name="xq", bufs=16) as xp,
                tc.tile_pool(name="qk_ps", bufs=4, space="PSUM") as qkps,
            ):
                for q in range(4):  # t-quarters of 1024
                    xq = [xp.tile([128, 1024], DT_MM, tag="xq", name=f"xq{q}_{i}") for i in range(8)]
                    dma_x = nc.sync if DT_MM == F32 else nc.gpsimd
                    for kt in range(8):
                        dma_x.dma_start(
                            xq[kt][:],
                            xT_d[kt * 128 : (kt + 1) * 128, q * 1024 : (q + 1) * 1024],
                        )
                    for ot, dest in enumerate((qT, kTt, vT)):
                        for th in range(2):  # 512-wide halves of the quarter
                            ps = qkps.tile([128, 512], F32, tag="qkps")
                            for kt in range(8):
                                nc.tensor.matmul(
                                    ps[:],
                                    wqkv_g[
                                            :,
                                            kt * 384 + ot * 128 : kt * 384 + (ot + 1) * 128,
                                        ]
                                    ,
                                    xq[kt][:, th * 512 : (th + 1) * 512],
                                    start=(kt == 0),
                                    stop=(kt == 7),
                                )
                            col = q * 1024 + th * 512
                            nc.vector.tensor_copy(dest[:, col : col + 512], ps[:])


            # ---------- Phase 2: attention ----------
            with (
                tc.tile_pool(name="es", bufs=10) as ep,
                tc.tile_pool(name="small", bufs=4) as sp,
                tc.tile_pool(name="unorm", bufs=4) as up,
                tc.tile_pool(name="os", bufs=6) as osp,
                tc.tile_pool(name="s_ps", bufs=4, space="PSUM") as sps,
                tc.tile_pool(name="pv_ps", bufs=2, space="PSUM") as pvps,
            ):
                # V^T [dv, t] -> V [t, dv] via PE transpose at the head of
                # phase 2 (s-tag psum slots) so the PE has dense work across
                # the phase boundary
                for jt in range(32):
                    ptv = sps.tile([128, 128], F32, tag="s", name=f"ptv{jt}")
                    nc.tensor.transpose(ptv[:], vT[:, jt * 128 : (jt + 1) * 128], ident[:])
                    nc.vector.tensor_copy(v1[:, jt * 65 : jt * 65 + 64], ptv[:, 0:64])
                    nc.vector.tensor_copy(v2[:, jt * 65 : jt * 65 + 64], ptv[:, 64:128])

                def emit_po(pb, pib, tt):
                    # out-projection matmuls for an already-normalized block
                    for oh in range(2):
                        tg = pib * 8 + tt
                        po = sps.tile([128, 512], F32, tag="s", name=f"po{pb}_{pib}_{tt}_{oh}")
                        nc.tensor.matmul(
                            po[:],
                            attnT[pb][:, tg * 128 : (tg + 1) * 128],
                            wo_g[:, oh * 512 : (oh + 1) * 512],
                            start=True,
                            stop=True,
                        )
                        ob = osp.tile([128, 512], F32, tag="ob", name=f"ob{pb}_{pib}_{tt}_{oh}")
                        if (tt + oh) % 2 == 0:
                            nc.vector.tensor_copy(ob[:], po[:])
                        else:
                            nc.scalar.copy(ob[:], po[:])
                        row = pb * 2048 + tg * 128
                        nc.sync.dma_start(
                            out_d[row : row + 128, oh * 512 : (oh + 1) * 512], ob[:]
                        )

                prev_block = None
                for b in range(B):
                    for ib in range(2):  # 1024-wide query blocks
                        i0 = b * 2048 + ib * 1024
                        pv = [pvps.tile([65, 1024], F32, tag="pv", name=f"pv{b}_{ib}_{i}") for i in range(2)]
                        for jt in range(16):  # 128-wide key tiles
                            j0 = b * 2048 + jt * 128
                            jv = (b * 16 + jt) * 65
                            # one 1-bank psum tile per (head, i-half): 4 slots ->
                            # deeper S->exp->PV pipeline keeps the PE array dense
                            s_h = [sps.tile([128, 512], F32, tag="s", name=f"s{b}_{ib}_{jt}_{i}") for i in range(4)]
                            e_h = [ep.tile([128, 512], DT_MM, tag="e", name=f"e{b}_{ib}_{jt}_{i}") for i in range(4)]
                            for h in range(2):
                                kTl = kTt[h * 64 : (h + 1) * 64, j0 : j0 + 128]
                                for ih in range(2):
                                    st = s_h[h * 2 + ih]
                                    nc.tensor.matmul(
                                        st[:],
                                        kTl,
                                        qT[
                                            h * 64 : (h + 1) * 64,
                                            i0 + ih * 512 : i0 + (ih + 1) * 512,
                                        ],
                                        start=True,
                                        stop=True,
                                        tile_position=(h * 64, 0),
                                    )
                                    nc.scalar.activation(
                                        e_h[h * 2 + ih][:], st[:], Exp, scale=SCALE
                                    )
                            for h, vv in enumerate((v1, v2)):
                                for ih in range(2):
                                    nc.tensor.matmul(
                                        pv[h][:, ih * 512 : (ih + 1) * 512],
                                        vv[:, jv : jv + 65],
                                        e_h[h * 2 + ih][:],
                                        start=(jt == 0),
                                        stop=(jt == 15),
                                    )
                            if prev_block is not None and jt % 2 == 1:
                                emit_po(prev_block[0], prev_block[1], jt // 2)
                        # --- normalization ---
                        # colsum rows live on psum partition 64; gather each to a
                        # partition-0 [1, 1024] tile (32-aligned reads/writes only)
                        cs_h = [sp.tile([1, 1024], F32, tag="cs", name=f"cs{b}_{ib}_{i}") for i in range(2)]
                        unorm = [up.tile([64, 1024], F32, tag="un", name=f"un{b}_{ib}_{i}") for i in range(2)]
                        for h in range(2):
                            nc.vector.tensor_copy(cs_h[h][:], pv[h][64:65, :])
                            # evacuate unnormalized attnout now so the pv psum
                            # banks free early; normalize later from SBUF
                            nc.vector.tensor_copy(unorm[h][:], pv[h][0:64, :])
                        # transpose 128-wide row chunks onto partitions: col c = blk*2+h
                        pt = pvps.tile([128, 16], F32, tag="pv")
                        for h in range(2):
                            for blk in range(8):
                                nc.tensor.transpose(
                                    pt[:, (blk * 2 + h) : (blk * 2 + h) + 1],
                                    cs_h[h][0:1, blk * 128 : (blk + 1) * 128],
                                    ident[0:1, 0:1],
                                )
                        cst = sp.tile([128, 16], F32, tag="cst")
                        nc.vector.tensor_copy(cst[:], pt[:])
                        rT = sp.tile([128, 16], F32, tag="rT")
                        nc.vector.reciprocal(rT[:], cst[:])
                        # transpose each column back to a [1, 128] row at partition 0
                        r2 = [sp.tile([1, 1024], DT_MM, tag="r2", name=f"r2_{b}_{ib}_{i}") for i in range(2)]
                        for h in range(2):
                            for blk in range(8):
                                c = blk * 2 + h
                                pr1 = pvps.tile([1, 128], F32, tag="pv", name=f"pr{b}_{ib}_{c}")
                                nc.tensor.transpose(pr1[:], rT[:, c : c + 1], ident[:])
                                nc.vector.tensor_copy(
                                    r2[h][0:1, blk * 128 : (blk + 1) * 128], pr1[:]
                                )
                        for h in range(2):
                            rbc = pvps.tile([64, 1024], F32, tag="pv")
                            for ih in range(2):
                                nc.tensor.matmul(
                                    rbc[:, ih * 512 : (ih + 1) * 512],
                                    ones1[:],
                                    r2[h][0:1, ih * 512 : (ih + 1) * 512],
                                    start=True,
                                    stop=True,
                                )
                            rbs = sp.tile([64, 1024], F32, tag="rbs")
                            nc.vector.tensor_copy(rbs[:], rbc[:])
                            nc.vector.tensor_tensor(
                                attnT[b][h * 64 : (h + 1) * 64, ib * 1024 : (ib + 1) * 1024],
                                unorm[h][:],
                                rbs[:],
                                mult,
                            )
                        prev_block = (b, ib)

                # flush the last block's out-projection
                for tt in range(8):
                    emit_po(prev_block[0], prev_block[1], tt)


    nc.compile()
    return nc


_NC = None


def _get_nc():
    global _NC
    if _NC is None:
        _NC = build_nc()
    return _NC


def _gate_pm1(mask):
    """Exact jax fp32 gate: sigmoid(m) > 0.5, encoded as +/-1 for device is_gt(0).

    Computed with the same fp32 logistic rounding as the reference (borderline
    tiny-positive m rounds sigmoid to exactly 0.5 -> gate False, unlike m > 0).
    """
    mask = np.asarray(mask, dtype=np.float32)
    g = (np.float32(1.0) / (np.float32(1.0) + np.exp(-mask))) > np.float32(0.5)
    return np.where(g, np.float32(1.0), np.float32(-1.0))


def make_in_maps(x, qkv_weight, qkv_weight_mask, out_weight, out_weight_mask):
    x = np.asarray(x, dtype=np.float32)
    qkv_weight = np.asarray(qkv_weight, dtype=np.float32)
    qkv_weight_mask = _gate_pm1(qkv_weight_mask)
    out_weight = np.asarray(out_weight, dtype=np.float32)
    out_weight_mask = _gate_pm1(out_weight_mask)

    xT = np.ascontiguousarray(x.reshape(T, DIM).T)
    in_maps = []
    for c in range(NCORES):
        r0 = c * DV  # 2c*64
        sl = slice(r0, r0 + DV)
        w_shard = np.concatenate(
            [qkv_weight[sl], qkv_weight[DIM + r0 : DIM + r0 + DV], qkv_weight[2 * DIM + r0 : 2 * DIM + r0 + DV]],
            axis=0,
        )  # [384, 1024] rows = (q h1,h2 | k h1,h2 | v h1,h2)
        m_shard = np.concatenate(
            [
                qkv_weight_mask[sl],
                qkv_weight_mask[DIM + r0 : DIM + r0 + DV],
                qkv_weight_mask[2 * DIM + r0 : 2 * DIM + r0 + DV],
            ],
            axis=0,
        )
        in_maps.append(
            {
                "xT": xT,
                "wqkvT": np.ascontiguousarray(w_shard.T),
                "mqkvT": np.ascontiguousarray(m_shard.T),
                "woT": np.ascontiguousarray(out_weight[:, sl].T),
                "moT": np.ascontiguousarray(out_weight_mask[:, sl].T),
            }
        )
    return in_maps


LAST_RESULTS = None  # BassKernelResults of the most recent run (for profiling)


def kernel(
    x,
    qkv_weight,
    qkv_weight_mask,
    out_weight,
    out_weight_mask,
    out_bias,
    out_bias_mask,
    _trace=False,
    _tmpdir=None,
):
    global LAST_RESULTS
    from concourse.bass_utils import run_bass_kernel_spmd

    nc = _get_nc()
    in_maps = make_in_maps(x, qkv_weight, qkv_weight_mask, out_weight, out_weight_mask)
    res = run_bass_kernel_spmd(
        nc, in_maps, list(range(NCORES)), trace=_trace, tmpdir=_tmpdir
    )
    LAST_RESULTS = res
    out = np.zeros((T, DIM), dtype=np.float32)
    for r in res.results:
        out += r["out"]
    out_bias = np.asarray(out_bias, dtype=np.float32)
    out_bias_mask = np.asarray(out_bias_mask, dtype=np.float32)
    out += np.where(_gate_pm1(out_bias_mask) > 0.0, out_bias, 0.0)[None, :]
    return out.reshape(B, N, DIM)

